# revision 1
# baseline (speedup 1.0000x reference)
"""Trainium2 Bass kernel for nn_BoxLoss (YOLO-style box regression loss), v3.

Contract: kernel(**inputs) takes FULL unsharded inputs (numpy), returns the
FULL scalar loss. Pure data parallel over batch across 8 NeuronCores (4
images per core); each core computes its 12 (scale, image) row losses
on-device and writes 2 partial sums; the host adds the 16 partials.

Layout: partition p = bh*50 + j (image-half, target), free col sbl =
s*2 + bl (scale, image-parity). The big [B,A,g,g,85] activations are
touched only via indirect gathers of the <=600 matched cells x 4 channels.

v3 vs baseline:
- anchor-derived constants (+-wh/2, areas) precomputed on host
- independent side ops (validity, anchor-union prep, cy*w) run on the
  otherwise-idle gpsimd engine in parallel with the vector chain
- gathers pair-ordered so each scale's stripe starts as soon as its two
  gathers land; dedup compare scheduled into the gather-wait gaps
- fused mask/key ops; final reduce fused via accum_out; [2,1] output
  summed on host
- dedup keeps the exact i16 DRAM-roundtrip broadcast (PE transpose has
  data on lhsT, which is exact in LOW_HIGH fp32)
"""

import numpy as np

import concourse.bass as bass
import concourse.bacc as bacc
import concourse.mybir as mybir
import concourse.tile as tile
from concourse.tile import add_dep_helper

NCORES = 8
GRIDS = (52, 26, 13)
A = 3
T = 50
PB = 4
SENT = 8112.0
B_TOTAL = 32
P100 = 2 * T
SBL = 6

F32 = mybir.dt.float32
I16 = mybir.dt.int16
I32 = mybir.dt.int32

_SCALE_ELEMS = [PB * A * g * g * 85 for g in GRIDS]
_SCALE_BASE = [0, _SCALE_ELEMS[0], _SCALE_ELEMS[0] + _SCALE_ELEMS[1]]
OUTCAT_ELEMS = sum(_SCALE_ELEMS)

# hostpack column layout ([100, _HP_TOT])
_H_TGT = 0        # [0,8)     raw targets (bl, c)
_H_G24 = 8        # [8,32)    g per (sbl, c)
_H_BG = 32        # [32,38)   scale base + b*3*g^2*85
_H_HW = 38        # [38,44)   g^2
_H_W = 44         # [44,50)   g
_H_AWHH = 50      # [50,86)   +anchor_wh/2 in (q, sbl, a)
_H_NAWHH = 86     # [86,122)  -anchor_wh/2
_H_AREAA = 122    # [122,140) anchor areas (sbl, a)
_H_ONESU = 140    # [140,142) bh-half indicators (matmul lhsT)
_HP_TOT = 142


def _host_consts():
    sbl = np.arange(SBL)
    s = sbl // 2
    g = np.array(GRIDS, dtype=np.float64)[s]              # [6]

    g24 = np.broadcast_to(g[:, None], (SBL, 4)).reshape(-1)       # [24]
    hw6 = g * g
    w6 = g
    p = np.arange(P100)
    bh = p // T
    base = np.array(_SCALE_BASE, dtype=np.float64)[s][None, :]
    b = (2 * bh[:, None] + (sbl % 2)[None, :])
    bg = base + b * (A * 85) * (g ** 2)[None, :]          # [100, 6]

    hp = np.zeros((P100, _HP_TOT), np.float64)
    hp[:, _H_G24:_H_G24 + 24] = g24[None, :]
    hp[:, _H_BG:_H_BG + 6] = bg
    hp[:, _H_HW:_H_HW + 6] = hw6[None, :]
    hp[:, _H_W:_H_W + 6] = w6[None, :]
    hp[0:T, _H_ONESU] = 1.0
    hp[T:P100, _H_ONESU + 1] = 1.0
    return hp.astype(np.float32)


def _inline_consts():
    # latks [100, 300] i16 in (k, s) layout: col = k*6 + s -> 1 if k > j
    j = (np.arange(P100) % T)[:, None]
    k = np.repeat(np.arange(T), SBL)[None, :]
    latks = (k > j).astype(np.int16)

    # L4 [4, 100]: rows (64*I_bh0, 64*I_bh1, I_bh0, I_bh1) for the exact
    # 64*hi + lo key-broadcast matmul
    l4 = np.zeros((4, P100), np.float32)
    l4[0, 0:T] = 64.0
    l4[1, T:P100] = 64.0
    l4[2, 0:T] = 1.0
    l4[3, T:P100] = 1.0
    return np.ascontiguousarray(latks), np.ascontiguousarray(l4)


def build_nc():
    nc = bacc.Bacc("TRN2", target_bir_lowering=False, debug=False,
                   num_devices=NCORES)

    hp_d = nc.dram_tensor("hostpack", [P100, _HP_TOT], F32, kind="ExternalInput")
    outcat_d = nc.dram_tensor("outcat", [OUTCAT_ELEMS], F32, kind="ExternalInput")
    loss_d = nc.dram_tensor("loss", [2, 1], F32, kind="ExternalOutput")
    lat2_np, l4_np = _inline_consts()
    lat2_d = nc.inline_tensor(lat2_np, name="latks")
    l4_d = nc.inline_tensor(l4_np, name="l4")

    AL = mybir.AluOpType
    AX = mybir.AxisListType.X

    with tile.TileContext(nc) as tc:
        with (
            tc.tile_pool(name="sbuf", bufs=1) as sp,
            tc.tile_pool(name="psum", bufs=1, space="PSUM") as pp,
        ):
            def tt(out, in0, in1, op):
                return nc.vector.tensor_tensor(out=out, in0=in0, in1=in1, op=op)

            def gtt(out, in0, in1, op):
                return nc.gpsimd.tensor_tensor(out=out, in0=in0, in1=in1, op=op)

            def ts(out, in0, s1, op, s2=None, op2=None):
                if op2 is None:
                    return nc.vector.tensor_scalar(out=out, in0=in0, scalar1=s1,
                                                   scalar2=None, op0=op)
                return nc.vector.tensor_scalar(out=out, in0=in0, scalar1=s1,
                                               scalar2=s2, op0=op, op1=op2)

            def stt(out, in0, scalar, in1, op0, op1, accum_out=None):
                return nc.vector.scalar_tensor_tensor(
                    out=out, in0=in0, scalar=scalar, in1=in1, op0=op0, op1=op1,
                    accum_out=accum_out)

            _tn = [0]

            def new(shape, dt=F32):
                _tn[0] += 1
                return sp.tile(shape, dt, name=f"t{_tn[0]}")

            # ---------- loads (hostpack split: chain cols land first) ----
            BF16 = mybir.dt.bfloat16
            hp = new([P100, _HP_TOT])
            nc.sync.dma_start(out=hp[:, 0:_H_AWHH], in_=hp_d[:, 0:_H_AWHH])
            nc.scalar.dma_start(out=hp[:, _H_AWHH:_HP_TOT],
                                in_=hp_d[:, _H_AWHH:_HP_TOT])
            lat = new([P100, SBL * T], I16)
            nc.scalar.dma_start(out=lat[:], in_=lat2_d[:, :])
            L4f = new([4, P100])
            nc.scalar.dma_start(out=L4f[:], in_=l4_d[:, :])
            L4b = new([4, P100], BF16)
            nc.vector.tensor_copy(out=L4b[:], in_=L4f[:])

            def C(c0, w):
                return hp[:, c0:c0 + w]

            tgt = C(_H_TGT, 8)
            tgv = tgt.rearrange("p (bl c) -> p bl c", c=4)
            onesU = C(_H_ONESU, 2)

            # ---------- t = raw * g ----------
            t4 = new([P100, 24])
            tt(t4[:], tgt[:, None, :].to_broadcast([P100, 3, 8]), C(_H_G24, 24),
               AL.mult)
            t4v = t4[:].rearrange("p (sbl c) -> p sbl c", c=4)
            txy = t4v[:, :, 0:2]
            twh = t4v[:, :, 2:4]


            # ---------- floor(xy) ----------
            r2 = new([P100, 12])
            ts(r2[:], txy, float(2 ** 23), AL.add, -float(2 ** 23), AL.add)
            gtm = new([P100, 12])
            tt(gtm[:], r2[:], txy, AL.is_gt)
            fxy = new([P100, 12])
            tt(fxy[:], r2[:], gtm[:], AL.subtract)
            fv = fxy[:].rearrange("p (sbl q) -> p sbl q", q=2)
            cx = fv[:, :, 0:1]
            cy = fv[:, :, 1:2]


            # ---------- target rect ----------
            zt05 = new([P100, 12])
            stt(zt05[:], txy, -0.5, fxy[:], AL.add, AL.subtract)
            lo = new([P100, 12])
            stt(lo[:], twh, -0.5, zt05[:], AL.mult, AL.add)
            hi = new([P100, 12])
            stt(hi[:], twh, 0.5, zt05[:], AL.mult, AL.add)

            # ---------- IoU ----------
            def bcQ(t12):
                return (t12[:].rearrange("p (sbl q) -> p q sbl", q=2)
                        [:, :, :, None].to_broadcast([P100, 2, SBL, 3]))

            P0 = new([P100, 36]); tt(P0[:], bcQ(lo), C(_H_NAWHH, 36), AL.max)
            P1 = new([P100, 36]); tt(P1[:], bcQ(hi), C(_H_AWHH, 36), AL.min)
            D = new([P100, 36]); tt(D[:], P1[:], P0[:], AL.subtract)
            M0 = new([P100, 36]); ts(M0[:], D[:], 0.0, AL.max)
            inter = new([P100, 18])
            tt(inter[:], M0[:, 0:18], M0[:, 18:36], AL.mult)
            areat = new([P100, 6])
            tt(areat[:], t4v[:, :, 2:3], t4v[:, :, 3:4], AL.mult)
            un1 = new([P100, 18])
            tt(un1[:], areat[:, :, None].to_broadcast([P100, SBL, 3]),
               C(_H_AREAA, 18), AL.add)
            union = new([P100, 18]); tt(union[:], un1[:], inter[:], AL.subtract)
            runi = new([P100, 18]); nc.vector.reciprocal(out=runi[:], in_=union[:])
            iou = new([P100, 18]); tt(iou[:], inter[:], runi[:], AL.mult)

            # ---------- overlap / argmax / cell / gather offsets ----------
            overlap = new([P100, 6])
            nc.vector.reduce_max(out=overlap[:],
                                 in_=iou[:].rearrange("p (sbl a) -> p sbl a", a=3),
                                 axis=AX)
            iv = iou[:].rearrange("p (sbl a) -> p sbl a", a=3)
            eqB = new([P100, 12])
            tt(eqB[:], iv[:, :, 0:2],
               overlap[:, :, None].to_broadcast([P100, SBL, 2]), AL.is_equal)
            ev = eqB[:].rearrange("p (sbl e) -> p sbl e", e=2)
            w1 = new([P100, 6])
            ts(w1[:], ev[:, :, 1:2], -2.0, AL.add)
            anc = new([P100, 6])
            stt(anc[:], ev[:, :, 0:1], -1.0, w1[:], AL.add, AL.mult)

            ca = new([P100, 6]); tt(ca[:], anc[:], C(_H_HW, 6), AL.mult)
            cb = new([P100, 6]); tt(cb[:], cy, C(_H_W, 6), AL.mult)
            cc = new([P100, 6]); tt(cc[:], ca[:], cb[:], AL.add)
            cell = new([P100, 6]); tt(cell[:], cc[:], cx, AL.add)
            idxi = new([P100, 6], I32)
            stt(idxi[:], cell[:], 85.0, C(_H_BG, 6), AL.mult, AL.add)

            # matched mask + dedup key (all-zero padding rows have IoU 0,
            # so the overlap threshold alone rejects them)
            m = new([P100, 6])
            ts(m[:], overlap[:], 0.5, AL.is_gt)
            kk = new([P100, 6])
            stt(kk[:], cell[:], -SENT, m[:], AL.add, AL.mult)
            # rsqrt of t_wh
            rwh2 = new([P100, 12])
            nc.vector.reciprocal(out=rwh2[:], in_=twh)
            rstw = new([P100, 12]); nc.scalar.sqrt(out=rstw[:], in_=rwh2[:])

            # ---------- indirect gathers (pair order: stripe s after 2s+1) --
            gall = new([P100, 24])   # (s, bl, c)
            gv6 = gall[:].rearrange("p (s bl c) -> p s bl c", bl=2, c=4)
            for q in range(6):
                s_, bl = q // 2, q % 2
                nc.gpsimd.indirect_dma_start(
                    out=gv6[:, s_, bl],
                    out_offset=None,
                    in_=outcat_d[:].unsqueeze(1),
                    in_offset=bass.IndirectOffsetOnAxis(ap=idxi[:, q:q + 1],
                                                        axis=0),
                )

            # ---------- dedup broadcast: exact bf16-split PE matmul --------
            # kk = 64*hi + lo with hi in [-127,0], lo in [0,64): both parts
            # are bf16-exact, so the K=4 bf16 matmul reconstructs kk exactly
            # in PSUM across all partitions -- no DRAM roundtrip.
            c64 = new([P100, 6]); ts(c64[:], kk[:], 1.0 / 64.0, AL.mult)
            r64 = new([P100, 6])
            ts(r64[:], c64[:], float(2 ** 23), AL.add, -float(2 ** 23), AL.add)
            g64 = new([P100, 6]); tt(g64[:], r64[:], c64[:], AL.is_gt)
            hiB = new([P100, 6], BF16)
            tt(hiB[:], r64[:], g64[:], AL.subtract)
            loB = new([P100, 6], BF16)
            stt(loB[:], hiB[:], -64.0, kk[:], AL.mult, AL.add)
            # relayout [100,6] -> one [1,300] row per (part, bh-half);
            # plain partition-range APs only (partition-dim splits in DMA
            # APs land elements wrong)
            rhs4 = new([4, SBL * T], BF16)
            nc.sync.dma_start(out=rhs4[0:1, :], in_=hiB[0:T, :])
            nc.sync.dma_start(out=rhs4[1:2, :], in_=hiB[T:P100, :])
            nc.scalar.dma_start(out=rhs4[2:3, :], in_=loB[0:T, :])
            nc.scalar.dma_start(out=rhs4[3:4, :], in_=loB[T:P100, :])
            keyB_p = pp.tile([P100, SBL * T], F32, name="keyB_p")
            nc.tensor.matmul(out=keyB_p[:], lhsT=L4b[:], rhs=rhs4[:],
                             start=True, stop=True)

            E = new([P100, SBL * T], I16)
            tt(E[:], kk[:, None, :].to_broadcast([P100, T, SBL]),
               keyB_p[:].rearrange("p (k s) -> p k s", s=SBL), AL.is_equal)
            EL = new([P100, SBL * T], I16)
            tt(EL[:], E[:], lat[:], AL.mult)
            ov = new([P100, 6], I16)
            nc.vector.reduce_max(out=ov[:],
                                 in_=EL[:].rearrange("p (k s) -> p s k", s=SBL),
                                 axis=AX)
            winner = new([P100, 6])
            _winner = stt(winner[:], ov[:], 0.0, m[:], AL.is_equal, AL.mult)
            # counts + denominator while the gathers are still in flight
            M1a_p = pp.tile([2, 6], F32, name="M1a_p")
            nc.tensor.matmul(out=M1a_p[:], lhsT=onesU, rhs=winner[:],
                             start=True, stop=True)
            mx2 = new([2, 6])
            ts(mx2[:], M1a_p[:], 1.0, AL.max, 2.0 * B_TOTAL, AL.mult)
            rden2 = new([2, 6]); nc.vector.reciprocal(out=rden2[:], in_=mx2[:])

            # ---------- stripe chains (per scale) ----------
            TS2 = new([P100, 6])
            for s_ in range(3):
                gv = gv6[:, s_]                      # [p, bl, c]
                t8 = t4v[:, 2 * s_:2 * s_ + 2, :]
                rcpw = new([P100, 4])
                _ri = nc.vector.reciprocal(out=rcpw[:], in_=gv[:, :, 2:4])
                if s_ == 2:
                    # dedup tail fits the gather-wait gap before stripe 2
                    add_dep_helper(_ri.ins, _winner.ins, True,
                                   "dedup before last stripe")
                rspw = new([P100, 4]); nc.scalar.sqrt(out=rspw[:], in_=rcpw[:])
                sel = new([P100, 8])
                selv = sel[:].rearrange("p (bl c) -> p bl c", c=4)
                _sxy = tt(selv[:, :, 0:2], gv[:, :, 0:2], t8[:, :, 0:2],
                          AL.subtract)
                if s_ == 2:
                    # keep stripe 2's gather-receipt wait (hoisted ahead of
                    # its consumers by the scheduler) from blocking the
                    # dedup tail: pin BOTH stripe-2 entry ops after winner
                    add_dep_helper(_sxy.ins, _winner.ins, True,
                                   "dedup before last stripe xy")
                tt(selv[:, :, 2:4], rspw[:], rstw[:, 4 * s_:4 * s_ + 4],
                   AL.subtract)
                sq = new([P100, 8]); tt(sq[:], sel[:], sel[:], AL.mult)
                nc.vector.reduce_sum(
                    out=TS2[:, 2 * s_:2 * s_ + 2],
                    in_=sq[:].rearrange("p (bl c) -> p bl c", c=4), axis=AX)

            # ---------- final reduction ----------
            wts = new([P100, 6])
            tt(wts[:], TS2[:], winner[:], AL.mult)
            M1b_p = pp.tile([2, 6], F32, name="M1b_p")
            nc.tensor.matmul(out=M1b_p[:], lhsT=onesU, rhs=wts[:],
                             start=True, stop=True)
            rl2 = new([2, 6])
            pt2 = new([2, 1])
            stt(rl2[:], M1b_p[:], 1.0, rden2[:], AL.mult, AL.mult,
                accum_out=pt2[:])
            nc.sync.dma_start(out=loss_d[:, :], in_=pt2[:])

    nc.compile()
    return nc


_HOST_CONSTS = _host_consts()


def make_in_maps(output0, anchors0, output1, anchors1, output2, anchors2,
                 targets):
    outs = [np.asarray(output0), np.asarray(output1), np.asarray(output2)]
    ancs = [np.asarray(anchors0), np.asarray(anchors1), np.asarray(anchors2)]
    tg = np.asarray(targets)

    awhh = np.zeros(36, np.float32)
    areaa = np.zeros(18, np.float32)
    for s_ in range(3):
        for bl in range(2):
            for a_ in range(3):
                col = (s_ * 2 + bl) * 3 + a_
                w_, h_ = float(ancs[s_][a_, 0]), float(ancs[s_][a_, 1])
                awhh[0 * 18 + col] = 0.5 * w_
                awhh[1 * 18 + col] = 0.5 * h_
                areaa[col] = w_ * h_

    in_maps = []
    for c in range(NCORES):
        sl = slice(c * PB, (c + 1) * PB)
        raw = tg[sl, :, 1:5].astype(np.float32)          # [4, 50, 4]
        tg8 = (raw.reshape(2, 2, T, 4)                    # (bh, bl, j, c)
               .transpose(0, 2, 1, 3).reshape(P100, 8))   # (bh,j) x (bl,c)
        hostpack = _HOST_CONSTS.copy()
        hostpack[:, _H_TGT:_H_TGT + 8] = tg8
        hostpack[:, _H_AWHH:_H_AWHH + 36] = awhh[None, :]
        hostpack[:, _H_NAWHH:_H_NAWHH + 36] = -awhh[None, :]
        hostpack[:, _H_AREAA:_H_AREAA + 18] = areaa[None, :]
        outcat = np.concatenate([o[sl].ravel() for o in outs]).astype(np.float32)
        in_maps.append({"hostpack": np.ascontiguousarray(hostpack),
                        "outcat": outcat})
    return in_maps


_NC_CACHE = {}


def kernel(output0, anchors0, output1, anchors1, output2, anchors2, targets):
    import time
    from concourse.bass_utils import run_bass_kernel_spmd

    if "nc" not in _NC_CACHE:
        _NC_CACHE["nc"] = build_nc()
    nc = _NC_CACHE["nc"]
    in_maps = make_in_maps(output0, anchors0, output1, anchors1, output2,
                           anchors2, targets)
    res = None
    for attempt in range(3):
        try:
            res = run_bass_kernel_spmd(nc, in_maps, list(range(NCORES)))
            break
        except Exception:
            if attempt == 2:
                raise
            time.sleep(20.0 * (attempt + 1))
    total = np.float32(0.0)
    for c in range(NCORES):
        total += np.float32(np.sum(res.results[c]["loss"]))
    return np.float32(total)



# revision 2
# speedup vs baseline: 1.0235x; 1.0235x over previous
"""Trainium2 Bass kernel for nn_BoxLoss (YOLO-style box regression loss), v5.

Contract: kernel(**inputs) takes FULL unsharded inputs (numpy), returns the
FULL scalar loss. Pure data parallel over batch across 8 NeuronCores (4
images per core); each core emits per-(target, scale-image-parity) weighted
square sums + winner mask [100, 12]; the host finishes the per-(image,scale)
normalization (divide by 2*max(n,1)*B) and the global sum.

v6 core idea: the gather needs only (image, scale, cell) -- NOT the argmax
anchor -- because the activations are relayed out host-side with the anchor
as an inner dim: outcat2[row, 0:12] = pred[b,:,cy,cx,0:4] for
row=(s,b,cy,cx), packed 12 f32 per row. The six indirect gathers (one per
(scale, image-parity) column; a single multi-offset DMA and dma_gather were
both falsified on HW -- garbage writes resp. ~20us ucode-library reloads)
therefore start right after the floor chain, overlapping the whole
IoU/argmax/dedup path. The anchor choice is applied post-gather by a
one-hot weighted reduce, pipelined per scale-stripe as gathers land.

Dedup (which targets survive last-writer-wins per (b,s,cell,anchor)):
PE transpose-of-broadcast of f32 keys (exact), then per-scale fused
compare*latermask with accum_out -> duplicate counts.

Layouts:
  box side:  partition p = bh*50 + j, free col sbl = s*2 + bl
  widx side: slot i = q*128 + p gives wrapped idx position (i%16, i//16),
             replicated every 16 partitions (host does the replication)
  gather out: gall3[p, q, 0:12] = (a,c) data for slot (p, q)
"""

import numpy as np

import concourse.bass as bass
import concourse.bacc as bacc
import concourse.mybir as mybir
import concourse.tile as tile

NCORES = 8
GRIDS = (52, 26, 13)
A = 3
T = 50
PB = 4
B_TOTAL = 32
P100 = 2 * T
SBL = 6

F32 = mybir.dt.float32
I16 = mybir.dt.int16
I32 = mybir.dt.int32

_G2 = [g * g for g in GRIDS]
_RBASE = [0, PB * _G2[0], PB * (_G2[0] + _G2[1])]
NROWS = PB * sum(_G2)          # 14196 gatherable rows per core

# hostpack column layout ([100, _HP_TOT])
_H_TGT = 0          # [0,8)      raw targets (bl, c)
_H_G24 = 8          # [8,32)     g per (sbl, c)
_H_G12N = 32        # [32,38)    -12*g
_H_RB12 = 38        # [38,44)    12*(rbase_s + b*g^2)
_D0 = 44            # end of chain-critical DMA0 block
_H_AWHH = 44        # [44,80)    +anchor_wh/2 in (q, sbl, a)
_H_NAWHH = 80       # [80,116)   -anchor_wh/2
_H_AREAA = 116      # [116,134)  anchor areas (sbl, a)
_H_C12 = 134        # [134,146)  (1,2) pattern per (sbl, e)
_HP_TOT = 146


def _host_consts():
    sbl = np.arange(SBL)
    s = sbl // 2
    g = np.array(GRIDS, dtype=np.float64)[s]              # [6]

    g24 = np.broadcast_to(g[:, None], (SBL, 4)).reshape(-1)       # [24]
    p = np.arange(P100)
    bh = p // T
    b = (2 * bh[:, None] + (sbl % 2)[None, :])            # [100, 6]
    rb = (np.array(_RBASE, dtype=np.float64)[s][None, :]
          + b * np.array(_G2, np.float64)[s][None, :])    # [100, 6]

    hp = np.zeros((P100, _HP_TOT), np.float64)
    hp[:, _H_G24:_H_G24 + 24] = g24[None, :]
    hp[:, _H_G12N:_H_G12N + 6] = (-12.0 * g)[None, :]
    hp[:, _H_RB12:_H_RB12 + 6] = 12.0 * rb
    c12 = np.broadcast_to(np.array([1.0, 2.0])[None, :], (SBL, 2)).reshape(-1)
    hp[:, _H_C12:_H_C12 + 12] = c12[None, :]
    return hp.astype(np.float32)


def _inline_consts():
    # [100, 200] f32: cols 0:100 identity, cols 100:200 latH
    p = np.arange(P100)
    ident = np.eye(P100, dtype=np.float32)
    same_half = (p[None, :] // T) == (p[:, None] // T)
    lat = (same_half & (p[None, :] > p[:, None])).astype(np.float32)
    return np.ascontiguousarray(
        np.concatenate([ident, lat], axis=1).astype(np.float32))


def build_nc():
    nc = bacc.Bacc("TRN2", target_bir_lowering=False, debug=False,
                   num_devices=NCORES)

    hp_d = nc.dram_tensor("hostpack", [P100, _HP_TOT], F32,
                          kind="ExternalInput")
    oc_d = nc.dram_tensor("outcat2", [NROWS * 12], F32, kind="ExternalInput")
    out_d = nc.dram_tensor("partial", [P100, 12], F32, kind="ExternalOutput")
    il_d = nc.inline_tensor(_inline_consts(), name="identlat")

    AL = mybir.AluOpType
    AX = mybir.AxisListType.X

    with tile.TileContext(nc) as tc:
        with (
            tc.tile_pool(name="sbuf", bufs=1) as sp,
            tc.tile_pool(name="psum", bufs=1, space="PSUM") as pp,
        ):
            def tt(out, in0, in1, op):
                return nc.vector.tensor_tensor(out=out, in0=in0, in1=in1, op=op)

            def gtt(out, in0, in1, op):
                return nc.gpsimd.tensor_tensor(out=out, in0=in0, in1=in1, op=op)

            def ts(out, in0, s1, op, s2=None, op2=None):
                if op2 is None:
                    return nc.vector.tensor_scalar(out=out, in0=in0, scalar1=s1,
                                                   scalar2=None, op0=op)
                return nc.vector.tensor_scalar(out=out, in0=in0, scalar1=s1,
                                               scalar2=s2, op0=op, op1=op2)

            def gts(out, in0, s1, op, s2=None, op2=None):
                if op2 is None:
                    return nc.gpsimd.tensor_scalar(out=out, in0=in0, scalar1=s1,
                                                   scalar2=None, op0=op)
                return nc.gpsimd.tensor_scalar(out=out, in0=in0, scalar1=s1,
                                               scalar2=s2, op0=op, op1=op2)

            def stt(out, in0, scalar, in1, op0, op1, accum_out=None):
                return nc.vector.scalar_tensor_tensor(
                    out=out, in0=in0, scalar=scalar, in1=in1, op0=op0, op1=op1,
                    accum_out=accum_out)

            def gstt(out, in0, scalar, in1, op0, op1):
                return nc.gpsimd.scalar_tensor_tensor(
                    out=out, in0=in0, scalar=scalar, in1=in1, op0=op0, op1=op1)

            _tn = [0]

            def new(shape, dt=F32):
                _tn[0] += 1
                return sp.tile(shape, dt, name=f"t{_tn[0]}")

            # ---------- loads ----------
            hp = new([P100, _HP_TOT])
            nc.sync.dma_start(out=hp[:, 0:_D0], in_=hp_d[:, 0:_D0])
            nc.scalar.dma_start(out=hp[:, _D0:_HP_TOT],
                                in_=hp_d[:, _D0:_HP_TOT])
            il = new([P100, 200])
            nc.gpsimd.dma_start(out=il[:], in_=il_d[:, :])
            ident = il[:, 0:100]
            latf = il[:, 100:200]

            def C(c0, w):
                return hp[:, c0:c0 + w]

            tgt = C(_H_TGT, 8)

            # ---------- DVE: box chain ----------
            t4 = new([P100, 24])
            tt(t4[:], tgt[:, None, :].to_broadcast([P100, 3, 8]), C(_H_G24, 24),
               AL.mult)
            t4v = t4[:].rearrange("p (sbl c) -> p sbl c", c=4)
            txy = t4v[:, :, 0:2]
            twh = t4v[:, :, 2:4]

            r2 = new([P100, 12])
            ts(r2[:], txy, float(2 ** 23), AL.add, -float(2 ** 23), AL.add)
            gtm = new([P100, 12])
            tt(gtm[:], r2[:], txy, AL.is_gt)
            z1 = new([P100, 12])
            stt(z1[:], txy, -0.5, r2[:], AL.add, AL.subtract)
            zt05 = new([P100, 12])
            tt(zt05[:], z1[:], gtm[:], AL.add)
            lo = new([P100, 12])
            stt(lo[:], twh, -0.5, zt05[:], AL.mult, AL.add)
            hi = new([P100, 12])
            stt(hi[:], twh, 0.5, zt05[:], AL.mult, AL.add)

            def bcQ(t12):
                return (t12[:].rearrange("p (sbl q) -> p q sbl", q=2)
                        [:, :, :, None].to_broadcast([P100, 2, SBL, 3]))

            P0 = new([P100, 36])
            tt(P0[:], bcQ(lo), C(_H_NAWHH, 36), AL.max)
            P1 = new([P100, 36])
            tt(P1[:], bcQ(hi), C(_H_AWHH, 36), AL.min)
            areat = new([P100, 6])
            tt(areat[:], t4v[:, :, 2:3], t4v[:, :, 3:4], AL.mult)
            un1 = new([P100, 18])
            tt(un1[:], areat[:, :, None].to_broadcast([P100, SBL, 3]),
               C(_H_AREAA, 18), AL.add)
            runi = new([P100, 18])
            nc.vector.reciprocal(out=runi[:], in_=un1[:])
            D = new([P100, 36])
            tt(D[:], P1[:], P0[:], AL.subtract)
            M0 = new([P100, 36])
            ts(M0[:], D[:], 0.0, AL.max)
            inter = new([P100, 18])
            tt(inter[:], M0[:, 0:18], M0[:, 18:36], AL.mult)
            v = new([P100, 18])
            tt(v[:], inter[:], runi[:], AL.mult)
            vv = v[:].rearrange("p (sbl a) -> p sbl a", a=3)
            vmax = new([P100, 6])
            nc.vector.reduce_max(out=vmax[:], in_=vv, axis=AX)
            eqB = new([P100, 12])
            tt(eqB[:], vv[:, :, 0:2],
               vmax[:, :, None].to_broadcast([P100, SBL, 2]), AL.is_equal)
            em = new([P100, 12])
            tt(em[:], eqB[:], C(_H_C12, 12), AL.subtract)
            emv = em[:].rearrange("p (sbl e) -> p sbl e", e=2)
            eqv = eqB[:].rearrange("p (sbl e) -> p sbl e", e=2)
            anc = new([P100, 6])
            tt(anc[:], emv[:, :, 0], emv[:, :, 1], AL.mult)

            # ---------- Pool: gather-row index chain + 6 indirect gathers --
            # row*12 = 12*rb + (12*cy)*g + 12*cx  (12 f32 per outcat2 row)
            nfxy = new([P100, 12])
            gtt(nfxy[:], gtm[:], r2[:], AL.subtract)   # = -floor(txy)
            nf12 = new([P100, 12])
            gts(nf12[:], nfxy[:], -12.0, AL.mult)      # = 12*floor(txy)
            nf12v = nf12[:].rearrange("p (sbl q) -> p sbl q", q=2)
            uy = new([P100, 6])
            gtt(uy[:], nf12v[:, :, 1],
                C(_H_G24, 24).rearrange("p (sbl c) -> p sbl c", c=4)[:, :, 0],
                AL.mult)
            s1g = new([P100, 6])
            gtt(s1g[:], uy[:], C(_H_RB12, 6), AL.add)
            i2 = new([P100, 6])
            gtt(i2[:], s1g[:], nf12v[:, :, 0], AL.add)
            idxi = new([P100, 6], I32)
            nc.gpsimd.tensor_copy(out=idxi[:], in_=i2[:])

            gall3 = new([P100, SBL * 12])
            gv3 = gall3[:].rearrange("p (q e) -> p q e", e=12)
            for q in range(SBL):
                nc.gpsimd.indirect_dma_start(
                    out=gv3[:, q],
                    out_offset=None,
                    in_=oc_d[:].unsqueeze(1),
                    in_offset=bass.IndirectOffsetOnAxis(
                        ap=idxi[:, q:q + 1], axis=0),
                )

            # ---------- DVE: mask, key, dedup ----------
            m6 = new([P100, 6])
            ts(m6[:], vmax[:], 1.0 / 3.0, AL.is_gt)
            cb3 = new([P100, 6])
            ts(cb3[:], i2[:], 0.25, AL.mult, 1.0, AL.add)   # = 3*row + 1
            k1 = new([P100, 6])
            tt(k1[:], anc[:], cb3[:], AL.add)
            kkey = new([P100, 6])
            tt(kkey[:], k1[:], m6[:], AL.mult)
            rwh2 = new([P100, 12])
            nc.vector.reciprocal(out=rwh2[:], in_=twh)
            rhs = new([P100, 24])
            rhsv = rhs[:].rearrange("p (q c) -> p q c", c=4)
            nc.vector.tensor_copy(out=rhsv[:, :, 0:2], in_=t4v[:, :, 0:2])
            # ACT: rstw into rhs wh slots (pulls the sqrt table load early)
            nc.scalar.sqrt(out=rhsv[:, :, 2:4],
                           in_=rwh2[:].rearrange("p (q c) -> p q c", c=2))

            keyB = pp.tile([P100, SBL, 128], F32, name="keyB")
            for s_ in range(SBL):
                nc.tensor.transpose(
                    keyB[:, s_, 0:P100],
                    kkey[:, s_:s_ + 1].to_broadcast([P100, P100]),
                    ident,
                )
            eo = new([P100, SBL * P100], I16)
            dups = new([P100, 6])
            outt = new([P100, 12])
            for s_ in range(SBL):
                stt(eo[:, s_ * P100:(s_ + 1) * P100],
                    keyB[:, s_, 0:P100],
                    kkey[:, s_:s_ + 1],
                    latf,
                    AL.is_equal, AL.mult,
                    accum_out=dups[:, s_:s_ + 1])
            winner = outt[:, 6:12]
            _winner = stt(winner, dups[:], 0.0, m6[:], AL.is_equal, AL.mult)

            # ---------- DVE: anchor one-hots ----------
            W = new([P100, 18])
            Wv = W[:].rearrange("p (sbl a) -> p sbl a", a=3)
            nem0 = new([P100, 6])
            ts(nem0[:], emv[:, :, 0], -1.0, AL.mult)   # = 1-e0
            ts(Wv[:, :, 0], emv[:, :, 0], 1.0, AL.add)  # = e0
            tt(Wv[:, :, 1], nem0[:], eqv[:, :, 1], AL.mult)
            tt(Wv[:, :, 2], nem0[:], Wv[:, :, 1], AL.subtract)

            # ---------- post-gather, pipelined per scale-stripe ----------
            from concourse.tile import add_dep_helper
            g3v = gall3[:].rearrange("p (q a c) -> p q a c", a=3, c=4)
            TSa = new([P100, 6])
            for st in range(3):
                q0 = 2 * st
                rcpw = new([P100, 12])
                _r = nc.vector.reciprocal(out=rcpw[:],
                                          in_=g3v[:, q0:q0 + 2, :, 2:4])
                # keep the scheduler from hoisting gather-gated stripe ops
                # ahead of the dedup chain in the in-order DVE stream
                add_dep_helper(_r.ins, _winner.ins, True,
                               f"dedup before stripe {st}")
                nc.scalar.sqrt(
                    out=g3v[:, q0:q0 + 2, :, 2:4],
                    in_=rcpw[:].rearrange("p (q a c) -> p q a c", a=3, c=2))
                sel = new([P100, 24])
                tt(sel[:], g3v[:, q0:q0 + 2],
                   rhsv[:, q0:q0 + 2, None, :].to_broadcast(
                       [P100, 2, 3, 4]), AL.subtract)
                sq = new([P100, 24])
                tt(sq[:], sel[:], sel[:], AL.mult)
                wsq = new([P100, 24])
                tt(wsq[:], sq[:],
                   Wv[:, q0:q0 + 2, :, None].to_broadcast([P100, 2, 3, 4]),
                   AL.mult)
                nc.vector.reduce_sum(
                    out=TSa[:, q0:q0 + 2],
                    in_=wsq[:].rearrange("p (q ac) -> p q ac", ac=12), axis=AX)
            tt(outt[:, 0:6], TSa[:], winner, AL.mult)

            nc.sync.dma_start(out=out_d[:, :], in_=outt[:])

    nc.compile()
    return nc


_HOST_CONSTS = _host_consts()


def make_in_maps(output0, anchors0, output1, anchors1, output2, anchors2,
                 targets):
    outs = [np.asarray(output0), np.asarray(output1), np.asarray(output2)]
    ancs = [np.asarray(anchors0), np.asarray(anchors1), np.asarray(anchors2)]
    tg = np.asarray(targets)

    awhh = np.zeros(36, np.float32)
    areaa = np.zeros(18, np.float32)
    for s_ in range(3):
        for bl in range(2):
            for a_ in range(3):
                col = (s_ * 2 + bl) * 3 + a_
                w_, h_ = float(ancs[s_][a_, 0]), float(ancs[s_][a_, 1])
                awhh[0 * 18 + col] = 0.5 * w_
                awhh[1 * 18 + col] = 0.5 * h_
                areaa[col] = w_ * h_

    in_maps = []
    for c in range(NCORES):
        sl = slice(c * PB, (c + 1) * PB)
        raw = tg[sl, :, 1:5].astype(np.float32)          # [4, 50, 4]
        tg8 = (raw.reshape(2, 2, T, 4)                    # (bh, bl, j, c)
               .transpose(0, 2, 1, 3).reshape(P100, 8))   # (bh,j) x (bl,c)
        hostpack = _HOST_CONSTS.copy()
        hostpack[:, _H_TGT:_H_TGT + 8] = tg8
        hostpack[:, _H_AWHH:_H_AWHH + 36] = awhh[None, :]
        hostpack[:, _H_NAWHH:_H_NAWHH + 36] = -awhh[None, :]
        hostpack[:, _H_AREAA:_H_AREAA + 18] = areaa[None, :]

        # anchor-inner gather table, packed 12 f32 per row
        oc2 = np.zeros((NROWS, 12), np.float32)
        for s_ in range(3):
            g = GRIDS[s_]
            blk = outs[s_][sl][:, :, :, :, 0:4]           # [4, 3, g, g, 4]
            t = np.transpose(blk, (0, 2, 3, 1, 4)).reshape(PB * g * g, 12)
            oc2[_RBASE[s_]:_RBASE[s_] + PB * g * g] = t

        in_maps.append({"hostpack": np.ascontiguousarray(hostpack),
                        "outcat2": np.ascontiguousarray(oc2.ravel())})
    return in_maps


def reduce_partials(partials):
    """partials: list of [100, 12] arrays (one per core) -> scalar loss."""
    total = 0.0
    for part in partials:
        part = np.asarray(part, np.float64)
        wts = part[:, 0:6].reshape(2, T, 6).sum(axis=1)   # [bh, sbl]
        n = part[:, 6:12].reshape(2, T, 6).sum(axis=1)    # [bh, sbl]
        denom = 2.0 * B_TOTAL * np.maximum(n, 1.0)
        total += float((wts / denom).sum())
    return np.float32(total)


_NC_CACHE = {}


def kernel(output0, anchors0, output1, anchors1, output2, anchors2, targets):
    import time
    from concourse.bass_utils import run_bass_kernel_spmd

    if "nc" not in _NC_CACHE:
        _NC_CACHE["nc"] = build_nc()
    nc = _NC_CACHE["nc"]
    in_maps = make_in_maps(output0, anchors0, output1, anchors1, output2,
                           anchors2, targets)
    res = None
    for attempt in range(3):
        try:
            res = run_bass_kernel_spmd(nc, in_maps, list(range(NCORES)))
            break
        except Exception:
            if attempt == 2:
                raise
            time.sleep(20.0 * (attempt + 1))
    return reduce_partials([res.results[c]["partial"] for c in range(NCORES)])
